# revision 5
# baseline (speedup 1.0000x reference)
"""Trainium2 Bass kernel for fp8-quantized dense matmul (dense_mlp).

Reference computation (per-tensor dynamic fp8 e4m3fn quantization):
    x:     [8, 8192, 512] f32  -> x2d [M=65536, K=512]
    w:     [512, 512] f32
    xs     = 448 / max(amax(|x|), 1e-12);  x_q = e4m3fn(x * xs)
    ws     = 448 / max(amax(|w|), 1e-12);  w_q = e4m3fn(w * ws)
    out    = (x_q @ w_q) * (1/xs) * (1/ws)          [M, 512] f32

Sharding: data-parallel over M across 8 cores (8192 rows each), weight
replicated; the x amax needs a cross-core AllReduce(max).

TRN2 fp8e4 (float8_e4m3) maxes out at +-240 (values in (240, 448] that OCP
e4m3fn can represent are Inf/NaN on TRN). We therefore quantize on-device
with scale' = 224/amax = (448/amax)/2. Scaling by an exact power of two
keeps every quantized value on the same relative grid, and the dequant
factor computed from the halved scales cancels exactly.

The kernel is PE-bound end to end, so both PE stages are minimized:
  - phase A casts x to f16 on DVE before the PE transposes: transpose-mode
    runs 1 cycle/row for 16-bit vs 2 for f32 (and HAM never upclocks
    transposes), halving ~55us of PE time to ~27us, under the 47us DMA.
  - phase B uses the (reused) weight as the stationary operand: consecutive
    DoubleRow matmuls share one LDWEIGHTS, instead of a fresh 256-column
    weight load per x-tile. Output lands transposed ([N, m]) and is
    re-transposed on the host during unsharding.

Precision (budget 2e-2; measured total ~4.5e-3): f16 quantizer input
~3.9e-3, f16 device output ~2e-4, DoubleRow pairing ~1e-4. The amax is
still computed from the raw f32 x, so the quantization grid matches the
reference exactly.
"""

from contextlib import nullcontext

import numpy as np

import concourse.bacc as bacc
import concourse.bass_isa as bass_isa
import concourse.mybir as mybir
import concourse.tile as tile
from concourse.bass_utils import run_bass_kernel_spmd
from concourse.masks import make_identity

F32 = mybir.dt.float32
F16 = mybir.dt.float16
FP8 = mybir.dt.float8e4

K = 512
N = 512
KB = K // 128  # k-blocks of 128 (partition-dim contraction tiles)
N_CORES = 8

# fp8 scale ceiling on TRN (e4m3 max normal is 240; 224 = 448/2 keeps the
# quantization grid exactly aligned with the reference's e4m3fn grid)
FP8_CEIL = 224.0


def build_nc(m_shard: int, n_cores: int = N_CORES,
             dma_chunk: int = 4, repeat: int = 1, phase_a_only: bool = False,
             layout: str = "wstat", cast_f16: bool = True,
             out_dtype=F16, xt_dtype=F16,
             store_chunk: int = 8, ostage_bufs: int = 3, dve_evac_every: int = 3,
             mw: int = 2048, use_doublerow: bool = True):
    """Build + compile the per-core SPMD program.

    m_shard: rows of x handled by this core (divisible by 128*dma_chunk)
    repeat: >1 builds a TIMING variant -- the x pipeline (phases A+B and the
        scale chain, minus the AllReduce, which cannot sit inside control
        flow) runs in a hardware For_i loop `repeat` times.
    layout: "wstat" = weight-stationary matmuls, transposed [N, m] output
        (host re-transposes); "xstat" = x-stationary, [m, N] output.
    """
    MT = m_shard // 128          # number of 128-row m-tiles
    CH = MT // dma_chunk         # number of DMA chunks

    nc = bacc.Bacc(
        trn_type="TRN2",
        target_bir_lowering=False,
        debug=False,
        num_devices=n_cores,
    )

    x_in = nc.dram_tensor("x", [m_shard, K], F32, kind="ExternalInput")
    w_in = nc.dram_tensor("w", [K, N], F32, kind="ExternalInput")
    out_shape = [N, m_shard] if layout == "wstat" else [m_shard, N]
    out_d = nc.dram_tensor("out", out_shape, out_dtype, kind="ExternalOutput")

    #  x rows (c*dma_chunk + j)*128 + p  ->  [c, p, j, k]
    x_re = x_in.ap().rearrange("(c j p) k -> c p j k", j=dma_chunk, p=128)
    #  w rows kb*128 + p -> [p, kb, n]
    w_re = w_in.ap().rearrange("(kb p) n -> p kb n", p=128)
    if layout == "wstat":
        out_re = out_d.ap().rearrange("(nb p) m -> nb p m", p=128)
    else:
        out_re = out_d.ap().rearrange("(c j p) n -> c p j n", j=store_chunk, p=128)

    tdt = F16 if cast_f16 else F32

    with tile.TileContext(nc) as tc:
        with (
            tc.tile_pool(name="pers", bufs=1) as pers,
            tc.tile_pool(name="xld", bufs=max(2, 16 // dma_chunk)) as xld,
            tc.tile_pool(name="xcp", bufs=3) as xcp,
            tc.tile_pool(name="xqp", bufs=8 if layout == "xstat" else 2) as xqp,
            tc.tile_pool(name="ostage", bufs=ostage_bufs) as ostage,
            tc.tile_pool(name="ps", bufs=2, space="PSUM") as ps,
            tc.tile_pool(name="ccdram", bufs=1, space="DRAM") as ccdram,
        ):
            # ---------------- persistent tiles ----------------
            ident = pers.tile([128, 128], F32)
            if cast_f16:
                identt = pers.tile([128, 128], tdt, name="identt")
            else:
                identt = ident
            w_f32 = pers.tile([128, KB, N], F32)
            wq = pers.tile([128, KB, N], FP8)
            xt_sb = pers.tile([128, KB, m_shard], xt_dtype)  # x^T (K on partitions)
            amax_slots = pers.tile([128, CH - 1 + dma_chunk], F32)

            def sc(name):
                return pers.tile([128, 1], F32, name=name)

            wa_part, wa_bc, wa_c, wa_r = sc("wa_part"), sc("wa_bc"), sc("wa_c"), sc("wa_r")
            xa_part, xa_bc, xa_g, xa_c, xa_r = (
                sc("xa_part"), sc("xa_bc"), sc("xa_g"), sc("xa_c"), sc("xa_r"))
            xsc, inv_xsc, dsc = sc("xsc"), sc("inv_xsc"), sc("dsc")

            make_identity(nc, ident)
            if cast_f16:
                make_identity(nc, identt)

            wpair = pers.tile([1, 2], F32, name="wpair")
            wsc_b = pers.tile([128, 2], F32, name="wsc_b")  # [wsc, 1/wsc] bcast

            def weight_path():
                # Quantize the (replicated) weight -- no collective needed.
                # Deliberately gpsimd-free: in the single-shot build this runs
                # during the x-amax AllReduce, and anything on gpsimd would
                # queue behind the collective's engine wait.
                nc.sync.dma_start(out=w_f32[:], in_=w_re)
                nc.vector.tensor_reduce(
                    out=wa_part[:], in_=w_f32[:], axis=mybir.AxisListType.XY,
                    op=mybir.AluOpType.max, apply_absolute_value=True,
                )
                wa_t = ps.tile([1, 128], F32, name="wa_t", tag="ps")
                nc.tensor.transpose(wa_t[:], wa_part[:], ident[:])
                nc.vector.tensor_reduce(
                    out=wa_bc[0:1, :], in_=wa_t[:], axis=mybir.AxisListType.X,
                    op=mybir.AluOpType.max,
                )
                nc.vector.tensor_scalar_max(wa_c[0:1, :], wa_bc[0:1, :], 1e-12)
                # wsc = 224 * (1/wa)  (TT divide is not a valid TRN2 DVE op)
                nc.vector.reciprocal(wa_r[0:1, :], wa_c[0:1, :])
                nc.vector.tensor_scalar_mul(wpair[:, 0:1], wa_r[0:1, :], FP8_CEIL)
                nc.vector.reciprocal(wpair[:, 1:2], wpair[:, 0:1])
                # broadcast [wsc, 1/wsc] to all 128 partitions: bounce the
                # 8B pair through DRAM, then re-read with a 0-stride
                # partition dim (exact; SBUF sources can't have 0-stride
                # partitions, DRAM sources can)
                wdram = ccdram.tile([1, 2], F32, name="wdram")
                nc.sync.dma_start(out=wdram[:], in_=wpair[:])
                nc.sync.dma_start(
                    out=wsc_b[:].rearrange("p (a b) -> p a b", a=1),
                    in_=wdram[:].partition_broadcast(128),
                )
                # quantize weight: wq = fp8(w * wsc)
                nc.scalar.mul(wq[:], w_f32[:], wsc_b[:, 0:1])

            # In timing builds the collective runs once, outside the loop
            # (collectives cannot appear inside control flow).
            timing_loop = repeat > 1
            if timing_loop:
                weight_path()
                weight_path = None
            if timing_loop and n_cores > 1:
                nc.vector.memset(xa_bc, 1.0)
                cc_in0 = ccdram.tile([128, 1], F32)
                cc_out0 = ccdram.tile([128, 1], F32)
                nc.gpsimd.dma_start(out=cc_in0[:], in_=xa_bc[:])
                nc.gpsimd.collective_compute(
                    "AllReduce",
                    mybir.AluOpType.max,
                    replica_groups=[list(range(n_cores))],
                    ins=[cc_in0.opt()],
                    outs=[cc_out0.opt()],
                )
                nc.gpsimd.dma_start(out=xa_g[:], in_=cc_out0[:])

            env = dict(
                nc=nc, tc=tc, CH=CH, dma_chunk=dma_chunk, MT=MT,
                x_re=x_re, out_re=out_re, xld=xld, xcp=xcp, xqp=xqp,
                ostage=ostage, ps=ps, ccdram=ccdram,
                ident=ident, identt=identt, tdt=tdt,
                w_f32=w_f32, wq=wq, xt_sb=xt_sb, amax_slots=amax_slots,
                xa_part=xa_part, xa_bc=xa_bc, xa_g=xa_g, xa_c=xa_c, xa_r=xa_r,
                xsc=xsc, inv_xsc=inv_xsc, wsc_b=wsc_b, dsc=dsc,
                n_cores=n_cores, cast_f16=cast_f16,
                out_dtype=out_dtype, dve_evac_every=dve_evac_every,
                use_doublerow=use_doublerow,
            )

            loop_cm = (
                tc.For_i(0, repeat, 1, hint_engines=(mybir.EngineType.PE,))
                if timing_loop else nullcontext()
            )
            with loop_cm:
                phase_a(**env)
                if not phase_a_only:
                    scale_chain(use_collective=not timing_loop,
                                weight_path=weight_path, **env)
                    if layout == "wstat":
                        phase_b_wstat(mw=mw, **env)
                    else:
                        phase_b_xstat(store_chunk=store_chunk, **env)

    nc.compile()
    return nc


def phase_a(nc, CH, dma_chunk, x_re, xld, xcp, ps, ident, identt, tdt,
            xt_sb, amax_slots, cast_f16, KB=KB, K_=K, **_):
    """Load x, abs-max (from f32), cast to f16 (DVE), transpose (PE)."""
    for c in range(CH):
        xt = xld.tile([128, dma_chunk, K_], F32)
        if c < CH - 1:
            nc.sync.dma_start(out=xt[:], in_=x_re[c])
            nc.vector.tensor_reduce(
                out=amax_slots[:, c:c + 1], in_=xt[:],
                axis=mybir.AxisListType.XY,
                op=mybir.AluOpType.max, apply_absolute_value=True,
            )
        else:
            # split the last chunk into per-m-tile DMAs + small amax ops so
            # the final abs-max lands right after the final (small) load
            for j in range(dma_chunk):
                nc.sync.dma_start(out=xt[:, j, :], in_=x_re[c, :, j, :])
                nc.vector.tensor_reduce(
                    out=amax_slots[:, c + j:c + j + 1], in_=xt[:, j, :],
                    axis=mybir.AxisListType.X,
                    op=mybir.AluOpType.max, apply_absolute_value=True,
                )
        if cast_f16:
            src = xcp.tile([128, dma_chunk, K_], tdt)
            nc.vector.tensor_copy(src[:], xt[:])
        else:
            src = xt
        for j2 in range(dma_chunk // 2):
            # two m-tiles per PSUM tile -> one FD-1024 evac
            tp = ps.tile([128, 2, KB, 128], tdt, tag="ps")
            for j in (2 * j2, 2 * j2 + 1):
                for kb in range(KB):
                    nc.tensor.transpose(
                        tp[:, j % 2, kb, :],
                        src[:, j, kb * 128:(kb + 1) * 128], identt[:],
                    )
            i = c * dma_chunk + 2 * j2   # first of the 2 m-tiles
            nc.scalar.copy(
                out=xt_sb[:, :, i * 128:(i + 2) * 128]
                .rearrange("p kb (j m) -> p j kb m", j=2),
                in_=tp[:],
            )


def scale_chain(nc, ccdram, amax_slots, xa_part, xa_bc, xa_g, xa_c, xa_r,
                xsc, n_cores, use_collective, weight_path, **_):
    """amax finalize, AllReduce(max), xsc. Emission order matters: the
    x-amax chain is emitted before the weight path on every engine."""
    nc.vector.tensor_reduce(
        out=xa_part[:], in_=amax_slots[:], axis=mybir.AxisListType.X,
        op=mybir.AluOpType.max,
    )
    nc.gpsimd.partition_all_reduce(
        xa_bc[:], xa_part[:], channels=128, reduce_op=bass_isa.ReduceOp.max,
    )
    cc_out = None
    if use_collective and n_cores > 1:
        cc_in = ccdram.tile([128, 1], F32)
        cc_out = ccdram.tile([128, 1], F32)
        nc.sync.dma_start(out=cc_in[:], in_=xa_bc[:])
        nc.gpsimd.collective_compute(
            "AllReduce",
            mybir.AluOpType.max,
            replica_groups=[list(range(n_cores))],
            ins=[cc_in.opt()],
            outs=[cc_out.opt()],
        )

    if weight_path is not None:
        # runs during the collective: the 1MB weight DMA + wq chain fill
        # the DMA/DVE/ACT gap instead of competing with phase A
        weight_path()

    if cc_out is not None:
        nc.sync.dma_start(out=xa_g[:], in_=cc_out[:])
    else:
        nc.vector.tensor_copy(xa_g[:], xa_bc[:])

    nc.vector.tensor_scalar_max(xa_c[:], xa_g[:], 1e-12)
    nc.vector.reciprocal(xa_r[:], xa_c[:])
    nc.vector.tensor_scalar_mul(xsc[:], xa_r[:], FP8_CEIL)


def _emit_dsc(nc, xsc, inv_xsc, dsc, wsc_b):
    # emitted after the first quantize: DVE executes in order, so placing
    # these between xsc and quantize_0 would delay the first matmul
    nc.vector.reciprocal(inv_xsc[:], xsc[:])
    nc.vector.tensor_tensor(
        out=dsc[:], in0=inv_xsc[:], in1=wsc_b[:, 1:2],
        op=mybir.AluOpType.mult,
    )


def phase_b_wstat(nc, MT, out_re, xqp, ostage, ps, wq, xt_sb,
                  xsc, inv_xsc, wsc_b, dsc, out_dtype, dve_evac_every,
                  use_doublerow, mw, KB=KB, N_=N, **_):
    """Weight-stationary matmuls: out^T[n, m] accumulates over kb pairs with
    the wq pair tile held stationary across a whole m-window sweep, so the
    256-column DoubleRow LDWEIGHTS is paid once per (window, nb, pair)
    instead of once per x-tile."""
    m_shard = MT * 128
    NW = m_shard // mw           # m-windows
    NB = N_ // 128               # output row blocks

    def quantize(w):
        xq = xqp.tile([128, KB, mw], FP8)
        nc.vector.tensor_scalar_mul(
            xq[:], xt_sb[:, :, w * mw:(w + 1) * mw], xsc[:],
        )
        return xq

    xq = quantize(0)
    _emit_dsc(nc, xsc, inv_xsc, dsc, wsc_b)
    u = 0
    for w in range(NW):
        xq_next = None
        for nb in range(NB):
            po = ps.tile([128, mw], F32, tag="ps")
            if use_doublerow:
                for kbi, kb in enumerate(range(0, KB, 2)):
                    for ms in range(mw // 512):
                        nc.tensor.matmul(
                            po[:, ms * 512:(ms + 1) * 512],
                            wq[:, kb:kb + 2, nb * 128:(nb + 1) * 128],
                            xq[:, kb:kb + 2, ms * 512:(ms + 1) * 512],
                            start=(kbi == 0), stop=(kbi == KB // 2 - 1),
                            perf_mode=mybir.MatmulPerfMode.DoubleRow,
                        )
            else:
                for kb in range(KB):
                    for ms in range(mw // 512):
                        nc.tensor.matmul(
                            po[:, ms * 512:(ms + 1) * 512],
                            wq[:, kb, nb * 128:(nb + 1) * 128],
                            xq[:, kb, ms * 512:(ms + 1) * 512],
                            start=(kb == 0), stop=(kb == KB - 1),
                        )
            if nb == 0 and w + 1 < NW:
                xq_next = quantize(w + 1)   # keep DVE a window ahead of PE
            ob = ostage.tile([128, mw], out_dtype)
            if dve_evac_every and u % dve_evac_every == dve_evac_every - 1:
                nc.vector.tensor_scalar_mul(ob[:], po[:], dsc[:])
            else:
                nc.scalar.mul(ob[:], po[:], dsc[:])
            nc.sync.dma_start(
                out=out_re[nb][:, w * mw:(w + 1) * mw], in_=ob[:],
            )
            u += 1
        if xq_next is not None:
            xq = xq_next


def phase_b_xstat(nc, MT, out_re, xqp, ostage, ps, wq, xt_sb,
                  xsc, inv_xsc, wsc_b, dsc, out_dtype, dve_evac_every,
                  use_doublerow, store_chunk, KB=KB, N_=N, **_):
    """x-stationary matmuls (original layout): out[m, N]."""
    PSC = 2                      # m-tiles per PSUM out tile (2 banks)
    NG = MT // PSC               # total PSUM groups
    GPC = store_chunk // PSC     # groups per store chunk

    def quantize(g):
        i0 = g * PSC
        xq = xqp.tile([128, KB, PSC * 128], FP8)
        nc.vector.tensor_scalar_mul(
            xq[:], xt_sb[:, :, i0 * 128:(i0 + PSC) * 128], xsc[:],
        )
        return xq

    xq_next = quantize(0)
    _emit_dsc(nc, xsc, inv_xsc, dsc, wsc_b)
    ob = None
    for g in range(NG):
        c, gi = divmod(g, GPC)
        if gi == 0:
            ob = ostage.tile([128, store_chunk, N_], out_dtype)
        po = ps.tile([128, PSC, N_], F32, tag="ps")
        xq2 = xq_next
        for j in range(PSC):
            xq_t = xq2[:, :, j * 128:(j + 1) * 128]
            if use_doublerow:
                for kb in range(0, KB, 2):
                    nc.tensor.matmul(
                        po[:, j, :], xq_t[:, kb:kb + 2, :],
                        wq[:, kb:kb + 2, :],
                        start=(kb == 0), stop=(kb == KB - 2),
                        perf_mode=mybir.MatmulPerfMode.DoubleRow,
                    )
            else:
                for kb in range(KB):
                    nc.tensor.matmul(
                        po[:, j, :], xq_t[:, kb, :], wq[:, kb, :],
                        start=(kb == 0), stop=(kb == KB - 1),
                    )
        if g + 1 < NG:
            xq_next = quantize(g + 1)
        dst = ob[:, gi * PSC:(gi + 1) * PSC, :]
        if dve_evac_every and g % dve_evac_every == dve_evac_every - 1:
            nc.vector.tensor_scalar_mul(dst, po[:], dsc[:])
        else:
            nc.scalar.mul(dst, po[:], dsc[:])
        if gi == GPC - 1:
            nc.sync.dma_start(out=out_re[c], in_=ob[:])


_CACHE: dict = {}


def _get_compiled(m_shard: int, **kw):
    key = (m_shard, tuple(sorted(kw.items())))
    if key not in _CACHE:
        _CACHE[key] = build_nc(m_shard, **kw)
    return _CACHE[key]


def run(x2d: np.ndarray, w: np.ndarray, trace: bool = False, **build_kw):
    """Run the SPMD kernel on [M, K] x and return ([M, N] f32 out, results)."""
    M = x2d.shape[0]
    assert M % N_CORES == 0
    m_shard = M // N_CORES
    nc = _get_compiled(m_shard, **build_kw)
    layout = build_kw.get("layout", "wstat")
    shards = x2d.reshape(N_CORES, m_shard, K)
    w = np.ascontiguousarray(w, dtype=np.float32)
    in_maps = [
        {"x": np.ascontiguousarray(shards[c]), "w": w} for c in range(N_CORES)
    ]
    res = run_bass_kernel_spmd(nc, in_maps, core_ids=list(range(N_CORES)),
                               trace=trace)
    if layout == "wstat":
        out = np.concatenate(
            [res.results[c]["out"].T for c in range(N_CORES)], axis=0)
    else:
        out = np.concatenate(
            [res.results[c]["out"] for c in range(N_CORES)], axis=0)
    return out.astype(np.float32), res


def kernel(x: np.ndarray, weight: np.ndarray) -> np.ndarray:
    x = np.asarray(x, dtype=np.float32)
    weight = np.asarray(weight, dtype=np.float32)
    B, S, k = x.shape
    assert k == K
    out, _ = run(x.reshape(-1, K), weight)
    return out.reshape(B, S, N).astype(np.float32)


# revision 11
# speedup vs baseline: 1.3255x; 1.3255x over previous
"""Trainium2 Bass kernel for fp8-quantized dense matmul (dense_mlp).

Reference computation (per-tensor dynamic fp8 e4m3fn quantization):
    x:     [8, 8192, 512] f32  -> x2d [M=65536, K=512]
    w:     [512, 512] f32
    xs     = 448 / max(amax(|x|), 1e-12);  x_q = e4m3fn(x * xs)
    ws     = 448 / max(amax(|w|), 1e-12);  w_q = e4m3fn(w * ws)
    out    = (x_q @ w_q) * (1/xs) * (1/ws)          [M, 512] f32

Sharding: data-parallel over M across 8 cores (8192 rows each), weight
replicated; the x amax needs a cross-core AllReduce(max).

TRN2 fp8e4 (float8_e4m3) maxes out at +-240 (values in (240, 448] that OCP
e4m3fn can represent are Inf/NaN on TRN). We therefore quantize on-device
with scale' = 224/amax = (448/amax)/2. Scaling by an exact power of two
keeps every quantized value on the same relative grid, and the dequant
factor computed from the halved scales cancels exactly.

Structure (measured on HW via For_i-loop slope; per 8192-row shard):
  phase A ~56us: stream 16 chunks of 1MiB x (DMA floor ~47us at 358GB/s),
    per-chunk DVE abs-max (f32, keeps the grid exact) and PE f32
    transposes into f16 xt_sb. Variants that pre-cast to f16 for cheaper
    transposes (DVE tensor_copy, any fraction) measured SLOWER -- the cast
    lengthens the per-chunk dependency chain more than it relieves PE.
    f32r transposes fail outright (f32r stationary is a known-broken path).
  scale chain ~14us exposed: partition_all_reduce (~1us) then an
    AllGather of one scalar per core + 0-stride-broadcast re-read + DVE
    max -- measured ~14us/unit vs ~25-35us for the ring AllReduce(max) the
    sharding hint suggests; AllToAll measured ~18us.
  phase B ~46us: PE-bound. x-tiles are the DoubleRow stationary operand
    (128 pair-matmuls of [256k x 128m] @ [256k x 512n] at ~420ns each --
    the 256-column DR LDWEIGHTS is serialized with its matmul; a probe
    with an identical stationary every matmul timed the same, so walrus
    does not skip redundant weight loads and weight-stationary sweeps
    cannot amortize them). Quantize (DVE 2x, f16-in) runs one PSUM group
    ahead of PE; dequant+evac splits ACT 2/3, DVE 1/3; f16 output stores
    (8 x 1MiB) hide under PE. Normal-mode fp8+FWL measured +25us; 2MiB or
    512KiB load chunks, deeper load buffers, and PSC/evac re-splits all
    measured neutral-or-worse.

Precision (budget 2e-2; measured total 4.5e-3): f16 quantizer input
~3.9e-3, f16 device output ~2e-4, DoubleRow pairing ~1e-4. The amax is
computed from the raw f32 x, so the quantization grid matches the
reference exactly.
"""

from contextlib import nullcontext

import numpy as np

import concourse.bacc as bacc
import concourse.bass_isa as bass_isa
import concourse.mybir as mybir
import concourse.tile as tile
from concourse.bass_utils import run_bass_kernel_spmd
from concourse.masks import make_identity

F32 = mybir.dt.float32
F16 = mybir.dt.float16
FP8 = mybir.dt.float8e4

K = 512
N = 512
KB = K // 128  # k-blocks of 128 (partition-dim contraction tiles)
N_CORES = 8

# fp8 scale ceiling on TRN (e4m3 max normal is 240; 224 = 448/2 keeps the
# quantization grid exactly aligned with the reference's e4m3fn grid)
FP8_CEIL = 224.0


def build_nc(m_shard: int, n_cores: int = N_CORES,
             dma_chunk: int = 4, repeat: int = 1, phase_a_only: bool = False,
             layout: str = "xstat", cast_mode: str = "none",
             out_dtype=F16, xt_dtype=F16,
             store_chunk: int = 8, ostage_bufs: int = 3, dve_evac_every: int = 3,
             mw: int = 2048, use_doublerow: bool = True,
             collective: str = "ag", fixed_stat: bool = False,
             xld_bufs: int | None = None, t_dtype: str = "f32",
             dr_mode: str = "dr"):
    """Build + compile the per-core SPMD program.

    m_shard: rows of x handled by this core (divisible by 128*dma_chunk)
    repeat: >1 builds a TIMING variant -- the x pipeline (phases A+B and the
        scale chain, minus the AllReduce, which cannot sit inside control
        flow) runs in a hardware For_i loop `repeat` times.
    layout: "wstat" = weight-stationary matmuls, transposed [N, m] output
        (host re-transposes); "xstat" = x-stationary, [m, N] output.
    """
    MT = m_shard // 128          # number of 128-row m-tiles
    CH = MT // dma_chunk         # number of DMA chunks

    nc = bacc.Bacc(
        trn_type="TRN2",
        target_bir_lowering=False,
        debug=False,
        num_devices=n_cores,
    )

    F32R = mybir.dt.float32r
    ldt = F32R if t_dtype == "f32r" else F32
    x_in = nc.dram_tensor("x", [m_shard, K], ldt, kind="ExternalInput")
    w_in = nc.dram_tensor("w", [K, N], F32, kind="ExternalInput")
    out_shape = [N, m_shard] if layout == "wstat" else [m_shard, N]
    out_d = nc.dram_tensor("out", out_shape, out_dtype, kind="ExternalOutput")

    #  x rows (c*dma_chunk + j)*128 + p  ->  [c, p, j, k]
    x_re = x_in.ap().rearrange("(c j p) k -> c p j k", j=dma_chunk, p=128)
    #  w rows kb*128 + p -> [p, kb, n]
    w_re = w_in.ap().rearrange("(kb p) n -> p kb n", p=128)
    if layout == "wstat":
        out_re = out_d.ap().rearrange("(nb p) m -> nb p m", p=128)
    else:
        out_re = out_d.ap().rearrange("(c j p) n -> c p j n", j=store_chunk, p=128)

    with tile.TileContext(nc) as tc:
        with (
            tc.tile_pool(name="pers", bufs=1) as pers,
            tc.tile_pool(name="xld", bufs=xld_bufs or max(2, 16 // dma_chunk)) as xld,
            tc.tile_pool(name="xcp", bufs=3) as xcp,
            tc.tile_pool(name="xqp", bufs=8 if layout == "xstat" else 2) as xqp,
            tc.tile_pool(name="ostage", bufs=ostage_bufs) as ostage,
            tc.tile_pool(name="ps", bufs=2, space="PSUM") as ps,
            tc.tile_pool(name="ccdram", bufs=1, space="DRAM") as ccdram,
        ):
            # ---------------- persistent tiles ----------------
            ident = pers.tile([128, 128], F32)
            identt = pers.tile([128, 128], F16, name="identt")
            if t_dtype == "f32r":
                identr = pers.tile([128, 128], F32R, name="identr")
                make_identity(nc, identr)
            else:
                identr = ident
            w_f32 = pers.tile([128, KB, N], F32)
            wq = pers.tile([128, KB, N], FP8)
            xt_sb = pers.tile([128, KB, m_shard], xt_dtype)  # x^T (K on partitions)
            amax_slots = pers.tile([128, CH - 1 + dma_chunk], F32)

            def sc(name):
                return pers.tile([128, 1], F32, name=name)

            wa_part, wa_bc, wa_c, wa_r = sc("wa_part"), sc("wa_bc"), sc("wa_c"), sc("wa_r")
            xa_part, xa_bc, xa_g, xa_c, xa_r = (
                sc("xa_part"), sc("xa_bc"), sc("xa_g"), sc("xa_c"), sc("xa_r"))
            xa_g8 = pers.tile([128, 8], F32, name="xa_g8")
            xsc, inv_xsc, dsc = sc("xsc"), sc("inv_xsc"), sc("dsc")

            make_identity(nc, ident)
            make_identity(nc, identt)

            wpair = pers.tile([1, 2], F32, name="wpair")
            wsc_b = pers.tile([128, 2], F32, name="wsc_b")  # [wsc, 1/wsc] bcast

            def weight_path():
                # Quantize the (replicated) weight -- no collective needed.
                # Deliberately gpsimd-free: in the single-shot build this runs
                # during the x-amax AllReduce, and anything on gpsimd would
                # queue behind the collective's engine wait.
                nc.sync.dma_start(out=w_f32[:], in_=w_re)
                nc.vector.tensor_reduce(
                    out=wa_part[:], in_=w_f32[:], axis=mybir.AxisListType.XY,
                    op=mybir.AluOpType.max, apply_absolute_value=True,
                )
                wa_t = ps.tile([1, 128], F32, name="wa_t", tag="ps")
                nc.tensor.transpose(wa_t[:], wa_part[:], ident[:])
                nc.vector.tensor_reduce(
                    out=wa_bc[0:1, :], in_=wa_t[:], axis=mybir.AxisListType.X,
                    op=mybir.AluOpType.max,
                )
                nc.vector.tensor_scalar_max(wa_c[0:1, :], wa_bc[0:1, :], 1e-12)
                # wsc = 224 * (1/wa)  (TT divide is not a valid TRN2 DVE op)
                nc.vector.reciprocal(wa_r[0:1, :], wa_c[0:1, :])
                nc.vector.tensor_scalar_mul(wpair[:, 0:1], wa_r[0:1, :], FP8_CEIL)
                nc.vector.reciprocal(wpair[:, 1:2], wpair[:, 0:1])
                # broadcast [wsc, 1/wsc] to all 128 partitions: bounce the
                # 8B pair through DRAM, then re-read with a 0-stride
                # partition dim (exact; SBUF sources can't have 0-stride
                # partitions, DRAM sources can)
                wdram = ccdram.tile([1, 2], F32, name="wdram")
                nc.sync.dma_start(out=wdram[:], in_=wpair[:])
                nc.sync.dma_start(
                    out=wsc_b[:].rearrange("p (a b) -> p a b", a=1),
                    in_=wdram[:].partition_broadcast(128),
                )
                # quantize weight: wq = fp8(w * wsc)
                nc.scalar.mul(wq[:], w_f32[:], wsc_b[:, 0:1])

            # In timing builds the collective runs once, outside the loop
            # (collectives cannot appear inside control flow).
            timing_loop = repeat > 1
            if timing_loop:
                weight_path()
                weight_path = None
            if timing_loop and n_cores > 1:
                nc.vector.memset(xa_bc, 1.0)
                cc_in0 = ccdram.tile([1, 1], F32)
                cc_out0 = ccdram.tile([1, n_cores], F32)
                nc.gpsimd.dma_start(out=cc_in0[:], in_=xa_bc[0:1, :])
                nc.gpsimd.collective_compute(
                    "AllGather",
                    mybir.AluOpType.bypass,
                    replica_groups=[list(range(n_cores))],
                    ins=[cc_in0.opt()],
                    outs=[cc_out0.opt()],
                )
                nc.gpsimd.dma_start(
                    out=xa_g8[0:1, 0:n_cores], in_=cc_out0[:])

            env = dict(
                nc=nc, tc=tc, CH=CH, dma_chunk=dma_chunk, MT=MT,
                x_re=x_re, out_re=out_re, xld=xld, xcp=xcp, xqp=xqp,
                ostage=ostage, ps=ps, ccdram=ccdram,
                ident=ident, identt=identt, identr=identr, ldt=ldt,
                w_f32=w_f32, wq=wq, xt_sb=xt_sb, amax_slots=amax_slots,
                xa_part=xa_part, xa_bc=xa_bc, xa_g=xa_g, xa_g8=xa_g8,
                xa_c=xa_c, xa_r=xa_r,
                xsc=xsc, inv_xsc=inv_xsc, wsc_b=wsc_b, dsc=dsc,
                n_cores=n_cores, cast_mode=cast_mode, collective=collective,
                out_dtype=out_dtype, dve_evac_every=dve_evac_every,
                use_doublerow=use_doublerow, fixed_stat=fixed_stat,
                dr_mode=dr_mode,
            )

            loop_cm = (
                tc.For_i(0, repeat, 1, hint_engines=(mybir.EngineType.PE,))
                if timing_loop else nullcontext()
            )
            with loop_cm:
                phase_a(**env)
                if not phase_a_only:
                    scale_chain(use_collective=not timing_loop,
                                weight_path=weight_path, **env)
                    if layout == "wstat":
                        phase_b_wstat(mw=mw, **env)
                    else:
                        phase_b_xstat(store_chunk=store_chunk, **env)

    nc.compile()
    return nc


def phase_a(nc, CH, dma_chunk, x_re, xld, xcp, ps, ident, identt, identr,
            ldt, xt_sb, amax_slots, cast_mode, KB=KB, K_=K, **_):
    """Load x, abs-max (from f32), cast to f16 (DVE, cast_mode chunks only),
    transpose (PE: 1 cyc/row for f16, 2 for f32)."""
    for c in range(CH):
        xt = xld.tile([128, dma_chunk, K_], ldt)
        if c < CH - 1:
            nc.sync.dma_start(out=xt[:], in_=x_re[c])
            nc.vector.tensor_reduce(
                out=amax_slots[:, c:c + 1], in_=xt[:],
                axis=mybir.AxisListType.XY,
                op=mybir.AluOpType.max, apply_absolute_value=True,
            )
        else:
            # split the last chunk into per-m-tile DMAs + small amax ops so
            # the final abs-max lands right after the final (small) load
            for j in range(dma_chunk):
                nc.sync.dma_start(out=xt[:, j, :], in_=x_re[c, :, j, :])
                nc.vector.tensor_reduce(
                    out=amax_slots[:, c + j:c + j + 1], in_=xt[:, j, :],
                    axis=mybir.AxisListType.X,
                    op=mybir.AluOpType.max, apply_absolute_value=True,
                )
        do_cast = cast_mode == "all" or (cast_mode == "half" and c % 2 == 0)
        if do_cast:
            src = xcp.tile([128, dma_chunk, K_], F16)
            nc.vector.tensor_copy(src[:], xt[:])
            idn, tdtc = identt, F16
        else:
            src, idn, tdtc = xt, identr, ldt
        for j2 in range(dma_chunk // 2):
            # two m-tiles per PSUM tile -> one FD-1024 evac
            tp = ps.tile([128, 2, KB, 128], tdtc, tag="ps")
            for j in (2 * j2, 2 * j2 + 1):
                for kb in range(KB):
                    nc.tensor.transpose(
                        tp[:, j % 2, kb, :],
                        src[:, j, kb * 128:(kb + 1) * 128], idn[:],
                    )
            i = c * dma_chunk + 2 * j2   # first of the 2 m-tiles
            nc.scalar.copy(
                out=xt_sb[:, :, i * 128:(i + 2) * 128]
                .rearrange("p kb (j m) -> p j kb m", j=2),
                in_=tp[:],
            )


def scale_chain(nc, ccdram, amax_slots, xa_part, xa_bc, xa_g, xa_g8, xa_c,
                xa_r, xsc, n_cores, use_collective, weight_path, collective,
                **_):
    """amax finalize, cross-core max (AllGather of scalars + local reduce is
    ~2.5x faster than ring AllReduce for 8 ranks), xsc. Emission order
    matters: the x-amax chain is emitted before the weight path on every
    engine."""
    nc.vector.tensor_reduce(
        out=xa_part[:], in_=amax_slots[:], axis=mybir.AxisListType.X,
        op=mybir.AluOpType.max,
    )
    nc.gpsimd.partition_all_reduce(
        xa_bc[:], xa_part[:], channels=128, reduce_op=bass_isa.ReduceOp.max,
    )
    cc_out = None
    if use_collective and n_cores > 1:
        if collective == "ag":
            cc_in = ccdram.tile([1, 1], F32)
            cc_out = ccdram.tile([1, n_cores], F32)
            nc.sync.dma_start(out=cc_in[:], in_=xa_bc[0:1, :])
            nc.gpsimd.collective_compute(
                "AllGather",
                mybir.AluOpType.bypass,
                replica_groups=[list(range(n_cores))],
                ins=[cc_in.opt()],
                outs=[cc_out.opt()],
            )
        else:
            cc_in = ccdram.tile([128, 1], F32)
            cc_out = ccdram.tile([128, 1], F32)
            nc.sync.dma_start(out=cc_in[:], in_=xa_bc[:])
            nc.gpsimd.collective_compute(
                "AllReduce",
                mybir.AluOpType.max,
                replica_groups=[list(range(n_cores))],
                ins=[cc_in.opt()],
                outs=[cc_out.opt()],
            )

    if weight_path is not None:
        # runs during the collective: the 1MB weight DMA + wq chain fill
        # the DMA/DVE/ACT gap instead of competing with phase A
        weight_path()

    if cc_out is not None:
        if collective == "ag":
            nc.sync.dma_start(
                out=xa_g8[:, 0:n_cores].rearrange("p (a b) -> p a b", a=1),
                in_=cc_out[:].partition_broadcast(128),
            )
            nc.vector.tensor_reduce(
                out=xa_g[:], in_=xa_g8[:, 0:n_cores],
                axis=mybir.AxisListType.X, op=mybir.AluOpType.max,
            )
        else:
            nc.sync.dma_start(out=xa_g[:], in_=cc_out[:])
    else:
        nc.vector.tensor_copy(xa_g[:], xa_bc[:])

    nc.vector.tensor_scalar_max(xa_c[:], xa_g[:], 1e-12)
    nc.vector.reciprocal(xa_r[:], xa_c[:])
    nc.vector.tensor_scalar_mul(xsc[:], xa_r[:], FP8_CEIL)


def _emit_dsc(nc, xsc, inv_xsc, dsc, wsc_b):
    # emitted after the first quantize: DVE executes in order, so placing
    # these between xsc and quantize_0 would delay the first matmul
    nc.vector.reciprocal(inv_xsc[:], xsc[:])
    nc.vector.tensor_tensor(
        out=dsc[:], in0=inv_xsc[:], in1=wsc_b[:, 1:2],
        op=mybir.AluOpType.mult,
    )


def phase_b_wstat(nc, MT, out_re, xqp, ostage, ps, wq, xt_sb,
                  xsc, inv_xsc, wsc_b, dsc, out_dtype, dve_evac_every,
                  use_doublerow, mw, KB=KB, N_=N, **_):
    """Weight-stationary matmuls: out^T[n, m] accumulates over kb pairs with
    the wq pair tile held stationary across a whole m-window sweep, so the
    256-column DoubleRow LDWEIGHTS is paid once per (window, nb, pair)
    instead of once per x-tile."""
    m_shard = MT * 128
    NW = m_shard // mw           # m-windows
    NB = N_ // 128               # output row blocks

    def quantize(w):
        xq = xqp.tile([128, KB, mw], FP8)
        nc.vector.tensor_scalar_mul(
            xq[:], xt_sb[:, :, w * mw:(w + 1) * mw], xsc[:],
        )
        return xq

    xq = quantize(0)
    _emit_dsc(nc, xsc, inv_xsc, dsc, wsc_b)
    u = 0
    for w in range(NW):
        xq_next = None
        for nb in range(NB):
            po = ps.tile([128, mw], F32, tag="ps")
            if use_doublerow:
                for kbi, kb in enumerate(range(0, KB, 2)):
                    for ms in range(mw // 512):
                        nc.tensor.matmul(
                            po[:, ms * 512:(ms + 1) * 512],
                            wq[:, kb:kb + 2, nb * 128:(nb + 1) * 128],
                            xq[:, kb:kb + 2, ms * 512:(ms + 1) * 512],
                            start=(kbi == 0), stop=(kbi == KB // 2 - 1),
                            perf_mode=mybir.MatmulPerfMode.DoubleRow,
                        )
            else:
                for kb in range(KB):
                    for ms in range(mw // 512):
                        nc.tensor.matmul(
                            po[:, ms * 512:(ms + 1) * 512],
                            wq[:, kb, nb * 128:(nb + 1) * 128],
                            xq[:, kb, ms * 512:(ms + 1) * 512],
                            start=(kb == 0), stop=(kb == KB - 1),
                        )
            if nb == 0 and w + 1 < NW:
                xq_next = quantize(w + 1)   # keep DVE a window ahead of PE
            ob = ostage.tile([128, mw], out_dtype)
            if dve_evac_every and u % dve_evac_every == dve_evac_every - 1:
                nc.vector.tensor_scalar_mul(ob[:], po[:], dsc[:])
            else:
                nc.scalar.mul(ob[:], po[:], dsc[:])
            nc.sync.dma_start(
                out=out_re[nb][:, w * mw:(w + 1) * mw], in_=ob[:],
            )
            u += 1
        if xq_next is not None:
            xq = xq_next


def phase_b_xstat(nc, MT, out_re, xqp, ostage, ps, wq, xt_sb,
                  xsc, inv_xsc, wsc_b, dsc, out_dtype, dve_evac_every,
                  use_doublerow, store_chunk, fixed_stat=False,
                  dr_mode="dr", KB=KB, N_=N, **_):
    """x-stationary matmuls (original layout): out[m, N]."""
    PSC = 2                      # m-tiles per PSUM out tile (2 banks)
    NG = MT // PSC               # total PSUM groups
    GPC = store_chunk // PSC     # groups per store chunk

    def quantize(g):
        i0 = g * PSC
        xq = xqp.tile([128, KB, PSC * 128], FP8)
        nc.vector.tensor_scalar_mul(
            xq[:], xt_sb[:, :, i0 * 128:(i0 + PSC) * 128], xsc[:],
        )
        return xq

    xq_next = quantize(0)
    _emit_dsc(nc, xsc, inv_xsc, dsc, wsc_b)
    ob = None
    for g in range(NG):
        c, gi = divmod(g, GPC)
        if gi == 0:
            ob = ostage.tile([128, store_chunk, N_], out_dtype)
        po = ps.tile([128, PSC, N_], F32, tag="ps")
        xq2 = xq_next
        for j in range(PSC):
            # fixed_stat: TIMING-ONLY probe -- every matmul loads the same
            # stationary tile to test whether walrus/PE skips redundant LDWs
            xq_t = (xq2[:, :, 0:128] if fixed_stat
                    else xq2[:, :, j * 128:(j + 1) * 128])
            if use_doublerow:
                pm = (mybir.MatmulPerfMode.DoubleRowSwInterleave
                      if dr_mode == "sw" else mybir.MatmulPerfMode.DoubleRow)
                for kb in range(0, KB, 2):
                    nc.tensor.matmul(
                        po[:, j, :], xq_t[:, kb:kb + 2, :],
                        wq[:, kb:kb + 2, :],
                        start=(kb == 0), stop=(kb == KB - 2),
                        perf_mode=pm,
                    )
            else:
                for kb in range(KB):
                    nc.tensor.matmul(
                        po[:, j, :], xq_t[:, kb, :], wq[:, kb, :],
                        start=(kb == 0), stop=(kb == KB - 1),
                    )
        if g + 1 < NG:
            xq_next = quantize(g + 1)
        dst = ob[:, gi * PSC:(gi + 1) * PSC, :]
        if dve_evac_every and g % dve_evac_every == dve_evac_every - 1:
            nc.vector.tensor_scalar_mul(dst, po[:], dsc[:])
        else:
            nc.scalar.mul(dst, po[:], dsc[:])
        if gi == GPC - 1:
            nc.sync.dma_start(out=out_re[c], in_=ob[:])


_CACHE: dict = {}


def _get_compiled(m_shard: int, **kw):
    key = (m_shard, tuple(sorted(kw.items())))
    if key not in _CACHE:
        _CACHE[key] = build_nc(m_shard, **kw)
    return _CACHE[key]


def run(x2d: np.ndarray, w: np.ndarray, trace: bool = False, **build_kw):
    """Run the SPMD kernel on [M, K] x and return ([M, N] f32 out, results)."""
    M = x2d.shape[0]
    assert M % N_CORES == 0
    m_shard = M // N_CORES
    nc = _get_compiled(m_shard, **build_kw)
    layout = build_kw.get("layout", "xstat")
    shards = x2d.reshape(N_CORES, m_shard, K)
    w = np.ascontiguousarray(w, dtype=np.float32)
    in_maps = [
        {"x": np.ascontiguousarray(shards[c]), "w": w} for c in range(N_CORES)
    ]
    res = run_bass_kernel_spmd(nc, in_maps, core_ids=list(range(N_CORES)),
                               trace=trace)
    if layout == "wstat":
        out = np.concatenate(
            [res.results[c]["out"].T for c in range(N_CORES)], axis=0)
    else:
        out = np.concatenate(
            [res.results[c]["out"] for c in range(N_CORES)], axis=0)
    return out.astype(np.float32), res


def kernel(x: np.ndarray, weight: np.ndarray) -> np.ndarray:
    x = np.asarray(x, dtype=np.float32)
    weight = np.asarray(weight, dtype=np.float32)
    B, S, k = x.shape
    assert k == K
    out, _ = run(x.reshape(-1, K), weight)
    return out.reshape(B, S, N).astype(np.float32)
